# revision 64
# baseline (speedup 1.0000x reference)
"""ViT-S/16 + LoRA forward pass on 8 Trainium2 NeuronCores.

Data-parallel over batch (2 images/core, weights replicated). The LoRA
factors are folded into the dense weights on the host (W_eff = W + 2*B@A,
mathematically exact), which removes the entire low-rank path from the
device program: ~48*T matmul cycles, 13 vector ops and 8 weight DMAs per
layer. On-device compute runs feature-major (activations stored
transposed, [feat, token]) which makes every matmul in the network a
natural PE op with zero on-chip transposes. fp32 data, fp32r (TF32-like)
tensor-engine matmuls at full PE rate, fp32 PSUM accumulation.

Self-contained: hardcodes all shapes from the problem spec.
"""

import sys

sys.path.insert(0, "/opt/trn_rl_repo")

from contextlib import ExitStack

import numpy as np

import concourse.bass as bass
import concourse.tile as tile
from concourse import bacc, mybir
from concourse import bass_utils

F32 = mybir.dt.float32
F32R = mybir.dt.float32r
AF = mybir.ActivationFunctionType
OP = mybir.AluOpType

# Model dims (from reference.py)
L, D, NH, HD, MLP, R = 12, 384, 6, 64, 1536, 128
P16, IMG, NPATCH, NTOK = 16, 384, 24, 577
B = 16
NCORES = 8
NI = B // NCORES          # images per core
T = NI * NTOK             # tokens per core (1154)
NPAT = NPATCH * NPATCH    # 576 patches per image
SCALING = 2.0
ATTN_SCALE = 1.0 / 8.0
EPS = 1e-6

FT = D // 128             # 3 feature tiles of the residual stream
QKT = (2 * D) // 128      # 6 out-tiles for q,k
FKT = MLP // 128          # 12 fc1 out-tiles
# token chunks for dense (all-token) phases; fp32r needs the moving dim
# even (it streams 2 fp32/cycle) and >= 256 for full rate
CH = [(0, 386), (386, 384), (770, 384)]
# patch-embed chunks (per image, 576 patches)
PCH = [(0, 288), (288, 288)]
# attention: n-chunks and m-tiles within one image (577 tokens)
ACH = [(0, 290), (289, 288)]  # col 289 computed twice (benign overlap)
# proj chunks aligned to image boundaries (all >= 256 moving)
CHI = [(0, 290), (289, 288), (577, 290), (866, 288)]
AMT = [(0, 128), (128, 128), (256, 128), (384, 128), (512, 65)]


def _pack_lhsT(w):
    """W [O, I] -> [O//128, 128(p of I-tile), I//128, 128(m)] so that
    tile[mt][p, kt, m] == W[mt*128+m, kt*128+p] (the [K, M] stationary
    operand for out = W @ x)."""
    o, i = w.shape
    return np.ascontiguousarray(
        w.reshape(o // 128, 128, i // 128, 128).transpose(0, 3, 2, 1)
    )


def _pack_rhs(w):
    """W [O, I] -> [128(p of I-tile), I//128, O] so that tile[p, kt, o]
    == W[o, kt*128+p] (feature-major rhs: rows = contraction dim)."""
    o, i = w.shape
    return np.ascontiguousarray(w.reshape(o, i // 128, 128).transpose(2, 1, 0))


def _host_prep(inputs):
    """Pure layout transforms of the full inputs into the DRAM layouts the
    device program consumes, with the LoRA factors folded into the dense
    weights (exact)."""
    f = np.float32
    inp = {k: np.asarray(v, f) for k, v in inputs.items()}

    d = {}
    # per-core image patches, feature-major rhs [core][128, 6, 2*576]
    img = inp["img"]
    patches = img.reshape(B, 3, NPATCH, P16, NPATCH, P16)
    patches = patches.transpose(0, 2, 4, 1, 3, 5).reshape(B, NPAT, 3 * P16 * P16)
    per_core_patches = []
    for c in range(NCORES):
        p = patches[c * NI:(c + 1) * NI].reshape(NI * NPAT, 768)
        per_core_patches.append(_pack_rhs(p))  # [128, 6, 1152]
    d["patches"] = per_core_patches

    d["patchw"] = _pack_lhsT(inp["patch_w"])                      # [3,128,6,128]
    d["pos"] = np.ascontiguousarray(
        inp["pos_embed"][0].reshape(NTOK, FT, 128).transpose(2, 1, 0)
    )                                                             # [128,3,577]
    d["cls"] = np.ascontiguousarray(
        inp["cls_token"][0, 0].reshape(FT, 128).T
    )                                                             # [128,3]

    def _group3(pk):
        """[6, 128, kt, 128] lhsT tiles -> [2, 128, kt, 384]: groups of 3
        M-tiles batched so one DMA loads one [128, kt, 384] tile."""
        mt6, p, kt, m = pk.shape
        g = pk.reshape(mt6 // 3, 3, p, kt, m).transpose(0, 2, 3, 1, 4)
        return np.ascontiguousarray(g.reshape(mt6 // 3, p, kt, 3 * m))

    # LoRA folded into the dense weights on host: W_eff = W + SCALING * B @ A.
    # Exact same function; removes the low-rank path from the device program.
    qkvw = inp["qkv_w"] + SCALING * np.einsum(
        "lor,lri->loi", inp["qkv_B"], inp["qkv_A"])
    projw = inp["proj_w"] + SCALING * np.einsum(
        "lor,lri->loi", inp["proj_B"], inp["proj_A"])
    fc1w = inp["fc1_w"] + SCALING * np.einsum(
        "lor,lri->loi", inp["fc1_B"], inp["fc1_A"])
    fc2w = inp["fc2_w"] + SCALING * np.einsum(
        "lor,lri->loi", inp["fc2_B"], inp["fc2_A"])

    d["qkvw"] = np.stack([_group3(_pack_lhsT(qkvw[l, : 2 * D])) for l in range(L)])
    d["qkvwv"] = np.stack([_pack_rhs(qkvw[l, 2 * D:]) for l in range(L)])

    d["projw"] = np.stack([_group3(_pack_lhsT(projw[l])) for l in range(L)])

    # fc1 weights resident per layer: [128(p), 3(kt), 1536(m)]
    d["fc1w"] = np.stack([
        np.ascontiguousarray(fc1w[l].reshape(MLP, FT, 128).transpose(2, 1, 0))
        for l in range(L)])

    # fc2 weights resident per layer: [128(p of MLP-tile), 12(kt), 384(m)]
    d["fc2w"] = np.stack([
        np.ascontiguousarray(fc2w[l].reshape(D, FKT, 128).transpose(2, 1, 0))
        for l in range(L)])

    d["headw1"] = _pack_lhsT(inp["head_w1"])       # [16,128,3,128]
    d["headw2"] = _pack_lhsT(inp["head_w2"])       # [2,128,16,128]
    d["ones"] = np.ones((128, 128), f)

    # ln scales/biases packed [128, L, FT] (only used when nontrivial)
    def _pack_ln(v):
        return np.ascontiguousarray(v.reshape(L, FT, 128).transpose(2, 0, 1))
    d["ln1s"], d["ln1b"] = _pack_ln(inp["ln1_s"]), _pack_ln(inp["ln1_b"])
    d["ln2s"], d["ln2b"] = _pack_ln(inp["ln2_s"]), _pack_ln(inp["ln2_b"])
    d["norms"] = np.ascontiguousarray(inp["norm_s"].reshape(FT, 128).T)
    d["normb"] = np.ascontiguousarray(inp["norm_b"].reshape(FT, 128).T)

    # triviality flags (fills in setup_inputs are ones/zeros)
    triv = dict(
        ln1=(np.all(inp["ln1_s"] == 1) and np.all(inp["ln1_b"] == 0)),
        ln2=(np.all(inp["ln2_s"] == 1) and np.all(inp["ln2_b"] == 0)),
        norm=(np.all(inp["norm_s"] == 1) and np.all(inp["norm_b"] == 0)),
        qkv_b=np.all(inp["qkv_b"] == 0), proj_b=np.all(inp["proj_b"] == 0),
        fc1_b=np.all(inp["fc1_b"] == 0), fc2_b=np.all(inp["fc2_b"] == 0),
        patch_b=np.all(inp["patch_b"] == 0),
        head_b1=np.all(inp["head_b1"] == 0), head_b2=np.all(inp["head_b2"] == 0),
    )
    if not all(triv.values()):
        # general path: per-feature biases packed for device use
        d["qkv_b"] = np.ascontiguousarray(inp["qkv_b"].reshape(L, 9, 128).transpose(2, 0, 1))
        d["qkv_bv"] = np.ascontiguousarray(inp["qkv_b"][:, 2 * D:].reshape(1, L, D))
        d["proj_b"] = np.ascontiguousarray(inp["proj_b"].reshape(L, FT, 128).transpose(2, 0, 1))
        d["fc1_b"] = np.ascontiguousarray(inp["fc1_b"].reshape(L, FKT, 128).transpose(2, 0, 1))
        d["fc2_b"] = np.ascontiguousarray(inp["fc2_b"].reshape(L, FT, 128).transpose(2, 0, 1))
        d["patch_b"] = np.ascontiguousarray(inp["patch_b"].reshape(FT, 128).T)
        d["head_b1"] = np.ascontiguousarray(inp["head_b1"].reshape(16, 128).T)
        d["head_b2"] = np.ascontiguousarray(inp["head_b2"].reshape(2, 128).T)
    return d, triv


def _build(triv, compile=True):
    """Emit + compile the Bass/Tile program (identical on all 8 cores)."""
    nc = bacc.Bacc("TRN2", target_bir_lowering=False, debug=False,
                   num_devices=NCORES)

    dr = {}

    def din(name, shape):
        dr[name] = nc.dram_tensor(name, list(shape), F32R, kind="ExternalInput")
        return dr[name]

    din("patches", (128, 6, NI * NPAT))
    din("patchw", (3, 128, 6, 128))
    din("pos", (128, FT, NTOK))
    din("cls", (128, FT))
    din("qkvw", (L, 2, 128, FT, 384))
    din("qkvwv", (L, 128, FT, D))
    din("projw", (L, 1, 128, FT, 384))
    din("fc1w", (L, 128, FT, MLP))
    din("fc2w", (L, 128, FKT, D))
    din("headw1", (16, 128, FT, 128))
    din("headw2", (2, 128, 16, 128))
    din("ones", (128, 128))
    if not triv["ln1"]:
        din("ln1s", (128, L, FT)); din("ln1b", (128, L, FT))
    if not triv["ln2"]:
        din("ln2s", (128, L, FT)); din("ln2b", (128, L, FT))
    if not triv["norm"]:
        din("norms", (128, FT)); din("normb", (128, FT))
    for bn, sh in [("qkv_b", (L, 9, 128)), ("proj_b", (L, FT, 128)),
                   ("fc1_b", (L, FKT, 128)), ("fc2_b", (L, FT, 128))]:
        if not triv[bn]:
            dr[bn] = nc.dram_tensor(bn, [128, sh[0], sh[1]], F32, kind="ExternalInput")
    if not triv["qkv_b"]:
        dr["qkv_bv"] = nc.dram_tensor("qkv_bv", [1, L, D], F32R, kind="ExternalInput")
    if not triv["patch_b"]:
        dr["patch_b"] = nc.dram_tensor("patch_b", [128, FT], F32, kind="ExternalInput")
    if not triv["head_b1"]:
        dr["head_b1"] = nc.dram_tensor("head_b1", [128, 16], F32, kind="ExternalInput")
    if not triv["head_b2"]:
        dr["head_b2"] = nc.dram_tensor("head_b2", [128, 2], F32, kind="ExternalInput")

    out_d = nc.dram_tensor("out", [2 * 128, NI], F32, kind="ExternalOutput")

    with tile.TileContext(nc) as tc, ExitStack() as ctx:
        # ---- persistent SBUF pools ----
        single = ctx.enter_context(tc.tile_pool(name="single", bufs=1))
        xpool = ctx.enter_context(tc.tile_pool(name="xres", bufs=2))
        hpool = ctx.enter_context(tc.tile_pool(name="hln", bufs=1))
        qkpool = ctx.enter_context(tc.tile_pool(name="qk", bufs=1))
        vpool = ctx.enter_context(tc.tile_pool(name="v", bufs=1))
        opool = ctx.enter_context(tc.tile_pool(name="oatt", bufs=1))
        ppool = ctx.enter_context(tc.tile_pool(name="pprob", bufs=6))
        statp = ctx.enter_context(tc.tile_pool(name="stat", bufs=1))
        sqpool = ctx.enter_context(tc.tile_pool(name="sq", bufs=3))
        srpool = ctx.enter_context(tc.tile_pool(name="sr", bufs=2))
        apool = ctx.enter_context(tc.tile_pool(name="agelu", bufs=2))
        w3pool = ctx.enter_context(tc.tile_pool(name="w3", bufs=2))
        bigw = ctx.enter_context(tc.tile_pool(name="bigw", bufs=1))
        bigw2 = ctx.enter_context(tc.tile_pool(name="bigw2", bufs=1))

        ones_sb = single.tile([128, 128], F32R, tag="ones")
        nc.sync.dma_start(out=ones_sb[:], in_=dr["ones"].ap())
        eps_sb = single.tile([128, 1], F32, tag="eps")
        nc.vector.memset(eps_sb[:], EPS)

        pos_sb = bigw.tile([128, FT, NTOK], F32R, tag="fc1w")
        nc.sync.dma_start(out=pos_sb[:], in_=dr["pos"].ap())
        cls_sb = single.tile([128, FT], F32R, tag="cls")
        nc.sync.dma_start(out=cls_sb[:], in_=dr["cls"].ap())

        lnS = {}
        if not triv["ln1"]:
            lnS["l1s"] = single.tile([128, L, FT], F32R, tag="l1s")
            lnS["l1b"] = single.tile([128, L, FT], F32R, tag="l1b")
            nc.sync.dma_start(out=lnS["l1s"][:], in_=dr["ln1s"].ap())
            nc.sync.dma_start(out=lnS["l1b"][:], in_=dr["ln1b"].ap())
        if not triv["ln2"]:
            lnS["l2s"] = single.tile([128, L, FT], F32R, tag="l2s")
            lnS["l2b"] = single.tile([128, L, FT], F32R, tag="l2b")
            nc.sync.dma_start(out=lnS["l2s"][:], in_=dr["ln2s"].ap())
            nc.sync.dma_start(out=lnS["l2b"][:], in_=dr["ln2b"].ap())
        biases = {}
        for bn, n1 in [("qkv_b", 9), ("proj_b", FT), ("fc1_b", FKT), ("fc2_b", FT)]:
            if not triv[bn]:
                biases[bn] = single.tile([128, L, n1], F32, tag=bn)
                nc.sync.dma_start(out=biases[bn][:], in_=dr[bn].ap())
        for bn, n1 in [("patch_b", FT), ("head_b1", 16), ("head_b2", 2)]:
            if not triv[bn]:
                biases[bn] = single.tile([128, n1], F32, tag=bn)
                nc.sync.dma_start(out=biases[bn][:], in_=dr[bn].ap())
        vb_sb = None
        if not triv["qkv_b"]:
            vb_sb = single.tile([1, L, D], F32R, tag="vb")
            nc.sync.dma_start(out=vb_sb[:], in_=dr["qkv_bv"].ap())

        def psum_copy(dst_ap, src_ap, bias_ap=None, eng=None):
            """PSUM -> SBUF move, optionally adding a per-partition bias."""
            if bias_ap is None:
                if eng == "act":
                    nc.scalar.copy(dst_ap, src_ap)
                else:
                    nc.vector.tensor_copy(dst_ap, src_ap)
            else:
                nc.vector.tensor_scalar_add(dst_ap, src_ap, bias_ap)

        # ---------------- patch embed + cls + pos ----------------
        x_t = xpool.tile([128, FT, T], F32R, tag="x")
        with tc.tile_pool(name="ps_patch", bufs=3, space="PSUM") as psp:
            pb = biases.get("patch_b")
            for i in range(NI):
                # cls token column
                nc.vector.tensor_tensor(
                    out=x_t[:, :, i * NTOK:i * NTOK + 1],
                    in0=cls_sb[:].unsqueeze(2),
                    in1=pos_sb[:, :, 0:1],
                    op=OP.add)
            for mt in range(FT):
                w = w3pool.tile([128, 6, 128], F32R, tag="wpatch", bufs=2,
                                name=f"patchw_{mt}")
                nc.sync.dma_start(out=w[:], in_=dr["patchw"].ap()[mt])
                for i in range(NI):
                    for (c0, csz) in PCH:
                        rhs = bigw2.tile([128, 6, csz], F32R, tag="fc2w",
                                         name=f"prhs_{mt}_{i}_{c0}")
                        nc.sync.dma_start(
                            out=rhs[:],
                            in_=dr["patches"].ap()[:, :, i * NPAT + c0:i * NPAT + c0 + csz])
                        ps = psp.tile([128, csz], F32, tag="mm")
                        for kt in range(6):
                            nc.tensor.matmul(ps[:], w[:, kt, :], rhs[:, kt, :],
                                             start=(kt == 0), stop=(kt == 5))
                        dst = x_t[:, mt, i * NTOK + 1 + c0:i * NTOK + 1 + c0 + csz]
                        pos_sl = pos_sb[:, mt, 1 + c0:1 + c0 + csz]
                        if pb is None:
                            nc.vector.tensor_tensor(out=dst, in0=ps[:], in1=pos_sl, op=OP.add)
                        else:
                            nc.vector.scalar_tensor_tensor(
                                out=dst, in0=ps[:], scalar=pb[:, mt], in1=pos_sl,
                                op0=OP.add, op1=OP.add)

        # ---------------- transformer layers ----------------
        _ln_uid = [0]

        def emit_ln(src, dst, s_ap, b_ap):
            """dst = LN(src) over the feature (partition-tiled) axis.
            src/dst: [128, FT, T] feature-major tiles. Fully chunk-granular so
            each chunk of dst unblocks downstream consumers early (cross-phase
            pipelining); stats via ones-matmuls (free 128-partition broadcast),
            squares on the otherwise-idle GPSIMD, rstd = exp(-0.5*ln(var))."""
            _ln_uid[0] += 1
            uid = _ln_uid[0]
            with tc.tile_pool(name="ps_ln", bufs=6, space="PSUM") as pln:
                m_b = statp.tile([128, T], F32, tag="m", name=f"lnm_{uid}")
                r_b = statp.tile([128, T], F32, tag="r", name=f"lnr_{uid}")
                for ci, (c0, csz) in enumerate(CH):
                    s1 = pln.tile([128, csz], F32, tag="ln", name=f"s1_{uid}_{ci}")
                    s2 = pln.tile([128, csz], F32, tag="ln", name=f"s2_{uid}_{ci}")
                    for ft in range(FT):
                        sl = src[:, ft, c0:c0 + csz]
                        sq = sqpool.tile([128, csz], F32R, tag="sq",
                                         name=f"sq_{uid}_{ci}_{ft}")
                        nc.gpsimd.tensor_mul(sq[:], sl, sl)
                        nc.tensor.matmul(s1[:], ones_sb[:], sl,
                                         start=(ft == 0), stop=(ft == FT - 1))
                        nc.tensor.matmul(s2[:], ones_sb[:], sq[:],
                                         start=(ft == 0), stop=(ft == FT - 1))
                    mc = m_b[:, c0:c0 + csz]
                    rc = r_b[:, c0:c0 + csz]
                    nc.vector.tensor_scalar_mul(mc, s1[:], 1.0 / D)
                    t2 = sqpool.tile([128, csz], F32, tag="sq", name=f"t2_{uid}_{ci}")
                    nc.gpsimd.tensor_mul(t2[:], mc, mc)
                    t1 = sqpool.tile([128, csz], F32, tag="sq", name=f"t1_{uid}_{ci}")
                    # var = s2/D - m^2; +eps folded into Ln's free bias
                    nc.vector.scalar_tensor_tensor(
                        out=t1[:], in0=s2[:], scalar=1.0 / D, in1=t2[:],
                        op0=OP.mult, op1=OP.subtract)
                    nc.scalar.activation(t1[:], t1[:], AF.Ln, bias=eps_sb[:])
                    nc.scalar.activation(rc, t1[:], AF.Exp, scale=-0.5)
                    for ft in range(FT):
                        dsl = dst[:, ft, c0:c0 + csz]
                        eng = nc.gpsimd if ft == 2 else nc.vector
                        eng.tensor_sub(dsl, src[:, ft, c0:c0 + csz], mc)
                        eng.tensor_mul(dsl, dsl, rc)
                        if s_ap is not None:
                            nc.vector.tensor_scalar(dsl, dsl, s_ap[:, ft], b_ap[:, ft],
                                                    op0=OP.mult, op1=OP.add)

        x_cur = x_t
        for l in range(L):
            # ---- LN1 ----
            h_t = hpool.tile([128, FT, T], F32R, tag="h")
            sA = lnS["l1s"][:, l, :] if not triv["ln1"] else None
            bA = lnS["l1b"][:, l, :] if not triv["ln1"] else None
            emit_ln(x_cur, h_t, sA, bA)

            # ---- qkv ----
            qk_t = qkpool.tile([128, QKT, T], F32R, tag="qk")
            v_t = vpool.tile([128, 2 * 5, NH, HD + 1], F32R, tag="v")
            with tc.tile_pool(name="ps_qkv", bufs=4, space="PSUM") as pq:
                qbias = biases.get("qkv_b")
                for g in range(2):
                    w = w3pool.tile([128, FT, 3 * 128], F32R, tag="w3b",
                                    bufs=2, name=f"qkvw_{l}_{g}")
                    nc.sync.dma_start(out=w[:], in_=dr["qkvw"].ap()[l, g])
                    for ms in range(3):
                        mt = g * 3 + ms
                        for (c0, csz) in CH:
                            ps = pq.tile([128, csz], F32, tag="mm")
                            for ft in range(FT):
                                nc.tensor.matmul(
                                    ps[:], w[:, ft, ms * 128:(ms + 1) * 128],
                                    h_t[:, ft, c0:c0 + csz],
                                    start=(ft == 0), stop=(ft == FT - 1))
                            psum_copy(qk_t[:, mt, c0:c0 + csz], ps[:],
                                      qbias[:, l, mt] if qbias is not None else None,
                                      eng=None)
                # v in token-major [tok, head, hd] with a fused ones column
                vr = w3pool.tile([128, FT, D], F32R, tag="vr", name=f"vr_{l}")
                nc.sync.dma_start(out=vr[:], in_=dr["qkvwv"].ap()[l])
                for i in range(NI):
                    for mi, (m0, msz) in enumerate(AMT):
                        g0 = i * NTOK + m0
                        ps = pq.tile([128, D], F32, tag="mm")
                        for ft in range(FT):
                            nc.tensor.matmul(ps[0:msz, :], h_t[:, ft, g0:g0 + msz],
                                             vr[:, ft, :], start=(ft == 0),
                                             stop=(ft == FT - 1 and vb_sb is None))
                        if vb_sb is not None:
                            nc.tensor.matmul(ps[0:msz, :], ones_sb[0:1, 0:msz],
                                             vb_sb[0:1, l, :], start=False, stop=True)
                        vdst = v_t[0:msz, i * 5 + mi, :, 0:HD]
                        vsrc = ps[0:msz, :].rearrange("p (h d) -> p h d", h=NH)
                        nc.vector.tensor_copy(vdst, vsrc)
                        nc.vector.tensor_copy(
                            v_t[0:msz, i * 5 + mi, :, HD:HD + 1],
                            ones_sb[0:msz, 0:NH].unsqueeze(2))

            # ---- attention ----
            # S^T computed per (img, head) into bank-aligned mega-PSUM tiles
            # ([128, 5 m-tiles, 512] + [128, 5, 65]) so exp is 4 ACT ops per
            # pair instead of 10, amortizing ACT's fixed per-op cost.
            o_t = opool.tile([128, FT, T], F32R, tag="o")
            with tc.tile_pool(name="ps_attn", bufs=1, space="PSUM") as pa, \
                 tc.tile_pool(name="ps_o", bufs=4, space="PSUM") as po:
                for i in range(NI):
                    for hh in range(NH):
                        qoff = 64 * (hh % 2)
                        qt = hh // 2
                        ktile = 3 + hh // 2
                        base = i * NTOK
                        # One [128, 2, 512] psum tile per m-tile holds both
                        # n-chunks bank-aligned: every matmul keeps a >=256
                        # moving dim (full fp32r rate) and S tiles pipeline
                        # at m-tile granularity (bufs=3).
                        pts = []
                        for mi, (m0, msz) in enumerate(AMT):
                            gm = base + m0
                            lhs = qk_t[qoff:qoff + HD, ktile, gm:gm + msz]
                            sps = pa.tile([128, 2, 512], F32, tag="s2", bufs=2,
                                          name=f"s_{l}_{i}_{hh}_{mi}")
                            for ci, (n0, nsz) in enumerate(ACH):
                                nc.tensor.matmul(
                                    sps[0:msz, ci, 0:nsz], lhs,
                                    qk_t[qoff:qoff + HD, qt,
                                         base + n0:base + n0 + nsz],
                                    start=True, stop=True)
                            pt = ppool.tile([128, 2, 290], F32R, tag="p", bufs=8,
                                            name=f"p_{l}_{i}_{hh}_{mi}")
                            # single exp over both chunks; the strided view's
                            # dead columns (slot 1, cols 288-289) are unread
                            nc.scalar.activation(pt[0:msz, :, :],
                                                 sps[0:msz, :, 0:290],
                                                 AF.Exp, scale=ATTN_SCALE)
                            pts.append(pt)
                        for ci, (n0, nsz) in enumerate(ACH):
                            gn = base + n0
                            ops = po.tile([128, nsz], F32, tag="o",
                                          name=f"ops_{l}_{i}_{hh}_{ci}")
                            for mi, (m0, msz) in enumerate(AMT):
                                nc.tensor.matmul(
                                    ops[0:HD + 1, :],
                                    v_t[0:msz, i * 5 + mi, hh, :],
                                    pts[mi][0:msz, ci, 0:nsz],
                                    start=(mi == 0), stop=(mi == len(AMT) - 1))
                            sr = srpool.tile([128, nsz], F32R, tag="sr",
                                             name=f"sr_{l}_{i}_{hh}_{ci}")
                            nc.vector.tensor_copy(sr[64:65, :], ops[64:65, :])
                            bc = po.tile([128, nsz], F32, tag="o",
                                         name=f"bc_{l}_{i}_{hh}_{ci}")
                            nc.tensor.matmul(bc[0:64, :], ones_sb[64:65, 0:64],
                                             sr[64:65, :], start=True, stop=True)
                            rec = srpool.tile([128, nsz], F32, tag="rec",
                                              name=f"rec_{l}_{i}_{hh}_{ci}")
                            nc.vector.reciprocal_approx_fast(
                                out=rec[0:64, :], in_=bc[0:64, :])
                            nc.vector.tensor_tensor(
                                out=o_t[qoff:qoff + HD, hh // 2, gn:gn + nsz],
                                in0=ops[0:HD, :], in1=rec[0:64, :], op=OP.mult)

            # ---- proj + residual ----
            x_new = xpool.tile([128, FT, T], F32R, tag="x")
            with tc.tile_pool(name="ps_proj", bufs=3, space="PSUM") as pp:
                pbias = biases.get("proj_b")
                wpj = w3pool.tile([128, FT, 3 * 128], F32R, tag="w3b",
                                  bufs=2, name=f"projw_{l}")
                nc.sync.dma_start(out=wpj[:], in_=dr["projw"].ap()[l, 0])
                for mt in range(FT):
                    for (c0, csz) in CHI:
                        ps = pp.tile([128, csz], F32, tag="mm")
                        for ft in range(FT):
                            nc.tensor.matmul(ps[:],
                                             wpj[:, ft, mt * 128:(mt + 1) * 128],
                                             o_t[:, ft, c0:c0 + csz],
                                             start=(ft == 0), stop=(ft == FT - 1))
                        dst = x_new[:, mt, c0:c0 + csz]
                        if pbias is None:
                            nc.vector.tensor_tensor(dst, ps[:],
                                                    x_cur[:, mt, c0:c0 + csz], op=OP.add)
                        else:
                            nc.vector.scalar_tensor_tensor(
                                out=dst, in0=ps[:], scalar=pbias[:, l, mt],
                                in1=x_cur[:, mt, c0:c0 + csz], op0=OP.add, op1=OP.add)
            x_cur = x_new

            # ---- LN2 ----
            h2 = hpool.tile([128, FT, T], F32R, tag="h")
            sA = lnS["l2s"][:, l, :] if not triv["ln2"] else None
            bA = lnS["l2b"][:, l, :] if not triv["ln2"] else None
            emit_ln(x_cur, h2, sA, bA)

            # ---- MLP: fc1 -> gelu -> fc2 + residual ----
            f1bias = biases.get("fc1_b")
            f2bias = biases.get("fc2_b")
            x_out = xpool.tile([128, FT, T], F32R, tag="x")
            with tc.tile_pool(name="ps_mlp", bufs=3, space="PSUM") as pm, \
                 tc.tile_pool(name="ps_f2", bufs=3, space="PSUM") as pf2:
                for (c0, csz) in CH:
                    accs = [pf2.tile([128, csz], F32, tag="f2acc",
                                     name=f"f2acc_{l}_{c0}_{mt}")
                            for mt in range(FT)]
                    for fg in range(FKT // 3):
                        f1wt = w3pool.tile([128, FT, 3 * 128], F32R, tag="w3b",
                                           bufs=2, name=f"f1w_{l}_{c0}_{fg}")
                        nc.sync.dma_start(
                            out=f1wt[:],
                            in_=dr["fc1w"].ap()[l][:, :, fg * 384:(fg + 1) * 384])
                        f2wt = w3pool.tile([128, 3, D], F32R, tag="fc2r",
                                           bufs=2, name=f"f2w_{l}_{c0}_{fg}")
                        nc.sync.dma_start(
                            out=f2wt[:],
                            in_=dr["fc2w"].ap()[l][:, fg * 3:(fg + 1) * 3, :])
                        for fs in range(3):
                            fk = fg * 3 + fs
                            f1ps = pm.tile([128, csz], F32, tag="fc1")
                            for ft in range(FT):
                                nc.tensor.matmul(f1ps[:],
                                                 f1wt[:, ft, fs * 128:(fs + 1) * 128],
                                                 h2[:, ft, c0:c0 + csz],
                                                 start=(ft == 0), stop=(ft == FT - 1))
                            a_t = apool.tile([128, csz], F32R, tag="a")
                            if f1bias is None:
                                nc.scalar.activation(a_t[:], f1ps[:], AF.Gelu)
                            else:
                                nc.scalar.activation(a_t[:], f1ps[:], AF.Gelu,
                                                     bias=f1bias[:, l, fk])
                            for mt in range(FT):
                                nc.tensor.matmul(accs[mt][:],
                                                 f2wt[:, fs, mt * 128:(mt + 1) * 128],
                                                 a_t[:], start=(fk == 0),
                                                 stop=(fk == FKT - 1))
                    for mt in range(FT):
                        dst = x_out[:, mt, c0:c0 + csz]
                        if f2bias is None:
                            nc.vector.tensor_tensor(dst, accs[mt][:],
                                                    x_cur[:, mt, c0:c0 + csz], op=OP.add)
                        else:
                            nc.vector.scalar_tensor_tensor(
                                out=dst, in0=accs[mt][:], scalar=f2bias[:, l, mt],
                                in1=x_cur[:, mt, c0:c0 + csz], op0=OP.add, op1=OP.add)
            x_cur = x_out

        # ---------------- final LN on cls columns + head ----------------
        # cls tokens are columns 0 and 577 of x
        cview = x_cur[:, :, :].rearrange("p f (i n) -> p f i n", n=NTOK)[:, :, :, 0]
        c_ln = single.tile([128, FT, NI], F32R, tag="cln")
        with tc.tile_pool(name="ps_fin", bufs=4, space="PSUM") as pf:
            s1 = pf.tile([128, NI], F32, tag="ln")
            s2 = pf.tile([128, NI], F32, tag="ln")
            sqc = single.tile([128, FT, NI], F32R, tag="sqc")
            for ft in range(FT):
                nc.scalar.activation(sqc[:, ft, :], cview[:, ft, :], AF.Square)
                nc.tensor.matmul(s1[:], ones_sb[:], cview[:, ft, :],
                                 start=(ft == 0), stop=(ft == FT - 1))
                nc.tensor.matmul(s2[:], ones_sb[:], sqc[:, ft, :],
                                 start=(ft == 0), stop=(ft == FT - 1))
            m_b = statp.tile([128, NI], F32, tag="m")
            nc.vector.tensor_scalar_mul(m_b[:], s1[:], 1.0 / D)
            t1 = statp.tile([128, NI], F32, tag="t1")
            nc.vector.tensor_scalar(t1[:], s2[:], 1.0 / D, EPS, op0=OP.mult, op1=OP.add)
            t2 = statp.tile([128, NI], F32, tag="r")
            nc.vector.tensor_mul(t2[:], m_b[:], m_b[:])
            nc.vector.tensor_sub(t1[:], t1[:], t2[:])
            nc.scalar.activation(t1[:], t1[:], AF.Ln)
            nc.scalar.activation(t1[:], t1[:], AF.Exp, scale=-0.5)
            for ft in range(FT):
                nc.vector.tensor_sub(c_ln[:, ft, :], cview[:, ft, :], m_b[:])
                nc.vector.tensor_mul(c_ln[:, ft, :], c_ln[:, ft, :], t1[:])
                if not triv["norm"]:
                    ns = single.tile([128, FT], F32R, tag="ns")
                    nb = single.tile([128, FT], F32R, tag="nb")
                    if ft == 0:
                        nc.sync.dma_start(out=ns[:], in_=dr["norms"].ap())
                        nc.sync.dma_start(out=nb[:], in_=dr["normb"].ap())
                    nc.vector.tensor_scalar(c_ln[:, ft, :], c_ln[:, ft, :],
                                            ns[:, ft], nb[:, ft],
                                            op0=OP.mult, op1=OP.add)

            # head: relu(w1 @ cls) -> w2 @ .
            h1_t = single.tile([128, 16, NI], F32R, tag="h1")
            hb1 = biases.get("head_b1")
            for mt in range(16):
                w = w3pool.tile([128, FT, 128], F32R, tag="w3")
                nc.sync.dma_start(out=w[:], in_=dr["headw1"].ap()[mt])
                ps = pf.tile([128, NI], F32, tag="hmm")
                for ft in range(FT):
                    nc.tensor.matmul(ps[:], w[:, ft, :], c_ln[:, ft, :],
                                     start=(ft == 0), stop=(ft == FT - 1))
                if hb1 is None:
                    nc.scalar.activation(h1_t[:, mt, :], ps[:], AF.Relu)
                else:
                    nc.scalar.activation(h1_t[:, mt, :], ps[:], AF.Relu,
                                         bias=hb1[:, mt])
            out_sb = single.tile([128, 2, NI], F32, tag="osb")
            hb2 = biases.get("head_b2")
            for mt in range(2):
                w2 = bigw.tile([128, 16, 128], F32R, tag="fc1w")
                nc.sync.dma_start(out=w2[:], in_=dr["headw2"].ap()[mt])
                ps = pf.tile([128, NI], F32, tag="hmm")
                for kt in range(16):
                    nc.tensor.matmul(ps[:], w2[:, kt, :], h1_t[:, kt, :],
                                     start=(kt == 0), stop=(kt == 15))
                psum_copy(out_sb[:, mt, :], ps[:],
                          hb2[:, mt] if hb2 is not None else None)
            nc.sync.dma_start(
                out=out_d.ap().rearrange("(mt p) c -> p mt c", p=128),
                in_=out_sb[:])

    if compile:
        nc.compile()
    return nc


_CACHE = {}


def _get_program(triv):
    key = tuple(sorted(triv.items()))
    if key not in _CACHE:
        _CACHE[key] = _build(triv)
    return _CACHE[key]


def kernel(**inputs) -> np.ndarray:
    d, triv = _host_prep(inputs)
    nc = _get_program(triv)

    common = {}
    for k in ("patchw", "pos", "cls", "qkvw", "qkvwv",
              "projw", "fc1w", "fc2w", "headw1", "headw2", "ones"):
        common[k] = d[k]
    if not triv["ln1"]:
        common["ln1s"], common["ln1b"] = d["ln1s"], d["ln1b"]
    if not triv["ln2"]:
        common["ln2s"], common["ln2b"] = d["ln2s"], d["ln2b"]
    if not triv["norm"]:
        common["norms"], common["normb"] = d["norms"], d["normb"]
    for bn in ("qkv_b", "proj_b", "fc1_b", "fc2_b", "patch_b", "head_b1", "head_b2"):
        if not triv[bn]:
            common[bn] = d[bn]
    if not triv["qkv_b"]:
        common["qkv_bv"] = d["qkv_bv"]

    in_maps = [dict(common, patches=d["patches"][c]) for c in range(NCORES)]
    res = bass_utils.run_bass_kernel_spmd(nc, in_maps, core_ids=list(range(NCORES)))

    out = np.zeros((B, 256), np.float32)
    for c in range(NCORES):
        oc = np.asarray(res.results[c]["out"], np.float32)   # [256, NI]
        out[c * NI:(c + 1) * NI, :] = oc.T
    return out


if __name__ == "__main__":
    import os, time
    triv = dict(ln1=True, ln2=True, norm=True, qkv_b=True, proj_b=True,
                fc1_b=True, fc2_b=True, patch_b=True, head_b1=True, head_b2=True)
    do_compile = os.environ.get("KERNEL_COMPILE", "0") == "1"
    t0 = time.time()
    nc = _build(triv, compile=do_compile)
    print("build s:", time.time() - t0, "compile:", do_compile)
    print("instructions:", sum(len(b.instructions) for b in nc.m.functions[0].blocks))
    from concourse.timeline_sim import TimelineSim
    ts = TimelineSim(nc, trace=False)
    dur = ts.simulate()
    print("TimelineSim duration:", dur, "ns")


# revision 66
# speedup vs baseline: 1.0182x; 1.0182x over previous
"""ViT-S/16 + LoRA forward pass on 8 Trainium2 NeuronCores.

Data-parallel over batch (2 images/core, weights replicated). The LoRA
factors are folded into the dense weights on the host (W_eff = W + 2*B@A,
mathematically exact), which removes the entire low-rank path from the
device program: ~48*T matmul cycles, 13 vector ops and 8 weight DMAs per
layer. On-device compute runs feature-major (activations stored
transposed, [feat, token]) which makes every matmul in the network a
natural PE op with zero on-chip transposes. fp32 data, fp32r (TF32-like)
tensor-engine matmuls at full PE rate, fp32 PSUM accumulation.

Self-contained: hardcodes all shapes from the problem spec.
"""

import sys

sys.path.insert(0, "/opt/trn_rl_repo")

from contextlib import ExitStack

import numpy as np

import concourse.bass as bass
import concourse.tile as tile
from concourse import bacc, mybir
from concourse import bass_utils

F32 = mybir.dt.float32
F32R = mybir.dt.float32r
AF = mybir.ActivationFunctionType
OP = mybir.AluOpType

# Model dims (from reference.py)
L, D, NH, HD, MLP, R = 12, 384, 6, 64, 1536, 128
P16, IMG, NPATCH, NTOK = 16, 384, 24, 577
B = 16
NCORES = 8
NI = B // NCORES          # images per core
T = NI * NTOK             # tokens per core (1154)
NPAT = NPATCH * NPATCH    # 576 patches per image
SCALING = 2.0
ATTN_SCALE = 1.0 / 8.0
EPS = 1e-6

FT = D // 128             # 3 feature tiles of the residual stream
QKT = (2 * D) // 128      # 6 out-tiles for q,k
FKT = MLP // 128          # 12 fc1 out-tiles
# token chunks for dense (all-token) phases; fp32r needs the moving dim
# even (it streams 2 fp32/cycle) and >= 256 for full rate
CH = [(0, 386), (386, 384), (770, 384)]
# patch-embed chunks (per image, 576 patches)
PCH = [(0, 288), (288, 288)]
# attention: n-chunks and m-tiles within one image (577 tokens)
ACH = [(0, 290), (289, 288)]  # col 289 computed twice (benign overlap)
# proj chunks aligned to image boundaries (all >= 256 moving)
CHI = [(0, 290), (289, 288), (577, 290), (866, 288)]
AMT = [(0, 128), (128, 128), (256, 128), (384, 128), (512, 65)]


def _pack_lhsT(w):
    """W [O, I] -> [O//128, 128(p of I-tile), I//128, 128(m)] so that
    tile[mt][p, kt, m] == W[mt*128+m, kt*128+p] (the [K, M] stationary
    operand for out = W @ x)."""
    o, i = w.shape
    return np.ascontiguousarray(
        w.reshape(o // 128, 128, i // 128, 128).transpose(0, 3, 2, 1)
    )


def _pack_rhs(w):
    """W [O, I] -> [128(p of I-tile), I//128, O] so that tile[p, kt, o]
    == W[o, kt*128+p] (feature-major rhs: rows = contraction dim)."""
    o, i = w.shape
    return np.ascontiguousarray(w.reshape(o, i // 128, 128).transpose(2, 1, 0))


def _host_prep(inputs):
    """Pure layout transforms of the full inputs into the DRAM layouts the
    device program consumes, with the LoRA factors folded into the dense
    weights (exact)."""
    f = np.float32
    inp = {k: np.asarray(v, f) for k, v in inputs.items()}

    d = {}
    # per-core image patches, feature-major rhs [core][128, 6, 2*576]
    img = inp["img"]
    patches = img.reshape(B, 3, NPATCH, P16, NPATCH, P16)
    patches = patches.transpose(0, 2, 4, 1, 3, 5).reshape(B, NPAT, 3 * P16 * P16)
    per_core_patches = []
    for c in range(NCORES):
        p = patches[c * NI:(c + 1) * NI].reshape(NI * NPAT, 768)
        per_core_patches.append(_pack_rhs(p))  # [128, 6, 1152]
    d["patches"] = per_core_patches

    d["patchw"] = _pack_lhsT(inp["patch_w"])                      # [3,128,6,128]
    d["pos"] = np.ascontiguousarray(
        inp["pos_embed"][0].reshape(NTOK, FT, 128).transpose(2, 1, 0)
    )                                                             # [128,3,577]
    d["cls"] = np.ascontiguousarray(
        inp["cls_token"][0, 0].reshape(FT, 128).T
    )                                                             # [128,3]

    def _group3(pk):
        """[6, 128, kt, 128] lhsT tiles -> [2, 128, kt, 384]: groups of 3
        M-tiles batched so one DMA loads one [128, kt, 384] tile."""
        mt6, p, kt, m = pk.shape
        g = pk.reshape(mt6 // 3, 3, p, kt, m).transpose(0, 2, 3, 1, 4)
        return np.ascontiguousarray(g.reshape(mt6 // 3, p, kt, 3 * m))

    # LoRA folded into the dense weights on host: W_eff = W + SCALING * B @ A.
    # Exact same function; removes the low-rank path from the device program.
    qkvw = inp["qkv_w"] + SCALING * np.einsum(
        "lor,lri->loi", inp["qkv_B"], inp["qkv_A"])
    projw = inp["proj_w"] + SCALING * np.einsum(
        "lor,lri->loi", inp["proj_B"], inp["proj_A"])
    fc1w = inp["fc1_w"] + SCALING * np.einsum(
        "lor,lri->loi", inp["fc1_B"], inp["fc1_A"])
    fc2w = inp["fc2_w"] + SCALING * np.einsum(
        "lor,lri->loi", inp["fc2_B"], inp["fc2_A"])

    d["qkvw"] = np.stack([_group3(_pack_lhsT(qkvw[l, : 2 * D])) for l in range(L)])
    d["qkvwv"] = np.stack([_pack_rhs(qkvw[l, 2 * D:]) for l in range(L)])

    d["projw"] = np.stack([_group3(_pack_lhsT(projw[l])) for l in range(L)])

    # fc1 weights resident per layer: [128(p), 3(kt), 1536(m)]
    d["fc1w"] = np.stack([
        np.ascontiguousarray(fc1w[l].reshape(MLP, FT, 128).transpose(2, 1, 0))
        for l in range(L)])

    # fc2 weights resident per layer: [128(p of MLP-tile), 12(kt), 384(m)]
    d["fc2w"] = np.stack([
        np.ascontiguousarray(fc2w[l].reshape(D, FKT, 128).transpose(2, 1, 0))
        for l in range(L)])

    d["headw1"] = _pack_lhsT(inp["head_w1"])       # [16,128,3,128]
    d["headw2"] = _pack_lhsT(inp["head_w2"])       # [2,128,16,128]
    d["ones"] = np.ones((128, 128), f)

    # ln scales/biases packed [128, L, FT] (only used when nontrivial)
    def _pack_ln(v):
        return np.ascontiguousarray(v.reshape(L, FT, 128).transpose(2, 0, 1))
    d["ln1s"], d["ln1b"] = _pack_ln(inp["ln1_s"]), _pack_ln(inp["ln1_b"])
    d["ln2s"], d["ln2b"] = _pack_ln(inp["ln2_s"]), _pack_ln(inp["ln2_b"])
    d["norms"] = np.ascontiguousarray(inp["norm_s"].reshape(FT, 128).T)
    d["normb"] = np.ascontiguousarray(inp["norm_b"].reshape(FT, 128).T)

    # triviality flags (fills in setup_inputs are ones/zeros)
    triv = dict(
        ln1=(np.all(inp["ln1_s"] == 1) and np.all(inp["ln1_b"] == 0)),
        ln2=(np.all(inp["ln2_s"] == 1) and np.all(inp["ln2_b"] == 0)),
        norm=(np.all(inp["norm_s"] == 1) and np.all(inp["norm_b"] == 0)),
        qkv_b=np.all(inp["qkv_b"] == 0), proj_b=np.all(inp["proj_b"] == 0),
        fc1_b=np.all(inp["fc1_b"] == 0), fc2_b=np.all(inp["fc2_b"] == 0),
        patch_b=np.all(inp["patch_b"] == 0),
        head_b1=np.all(inp["head_b1"] == 0), head_b2=np.all(inp["head_b2"] == 0),
    )
    if not all(triv.values()):
        # general path: per-feature biases packed for device use
        d["qkv_b"] = np.ascontiguousarray(inp["qkv_b"].reshape(L, 9, 128).transpose(2, 0, 1))
        d["qkv_bv"] = np.ascontiguousarray(inp["qkv_b"][:, 2 * D:].reshape(1, L, D))
        d["proj_b"] = np.ascontiguousarray(inp["proj_b"].reshape(L, FT, 128).transpose(2, 0, 1))
        d["fc1_b"] = np.ascontiguousarray(inp["fc1_b"].reshape(L, FKT, 128).transpose(2, 0, 1))
        d["fc2_b"] = np.ascontiguousarray(inp["fc2_b"].reshape(L, FT, 128).transpose(2, 0, 1))
        d["patch_b"] = np.ascontiguousarray(inp["patch_b"].reshape(FT, 128).T)
        d["head_b1"] = np.ascontiguousarray(inp["head_b1"].reshape(16, 128).T)
        d["head_b2"] = np.ascontiguousarray(inp["head_b2"].reshape(2, 128).T)
    return d, triv


def _build(triv, compile=True):
    """Emit + compile the Bass/Tile program (identical on all 8 cores)."""
    nc = bacc.Bacc("TRN2", target_bir_lowering=False, debug=False,
                   num_devices=NCORES)

    dr = {}

    def din(name, shape):
        dr[name] = nc.dram_tensor(name, list(shape), F32R, kind="ExternalInput")
        return dr[name]

    din("patches", (128, 6, NI * NPAT))
    din("patchw", (3, 128, 6, 128))
    din("pos", (128, FT, NTOK))
    din("cls", (128, FT))
    din("qkvw", (L, 2, 128, FT, 384))
    din("qkvwv", (L, 128, FT, D))
    din("projw", (L, 1, 128, FT, 384))
    din("fc1w", (L, 128, FT, MLP))
    din("fc2w", (L, 128, FKT, D))
    din("headw1", (16, 128, FT, 128))
    din("headw2", (2, 128, 16, 128))
    din("ones", (128, 128))
    if not triv["ln1"]:
        din("ln1s", (128, L, FT)); din("ln1b", (128, L, FT))
    if not triv["ln2"]:
        din("ln2s", (128, L, FT)); din("ln2b", (128, L, FT))
    if not triv["norm"]:
        din("norms", (128, FT)); din("normb", (128, FT))
    for bn, sh in [("qkv_b", (L, 9, 128)), ("proj_b", (L, FT, 128)),
                   ("fc1_b", (L, FKT, 128)), ("fc2_b", (L, FT, 128))]:
        if not triv[bn]:
            dr[bn] = nc.dram_tensor(bn, [128, sh[0], sh[1]], F32, kind="ExternalInput")
    if not triv["qkv_b"]:
        dr["qkv_bv"] = nc.dram_tensor("qkv_bv", [1, L, D], F32R, kind="ExternalInput")
    if not triv["patch_b"]:
        dr["patch_b"] = nc.dram_tensor("patch_b", [128, FT], F32, kind="ExternalInput")
    if not triv["head_b1"]:
        dr["head_b1"] = nc.dram_tensor("head_b1", [128, 16], F32, kind="ExternalInput")
    if not triv["head_b2"]:
        dr["head_b2"] = nc.dram_tensor("head_b2", [128, 2], F32, kind="ExternalInput")

    out_d = nc.dram_tensor("out", [2 * 128, NI], F32, kind="ExternalOutput")

    with tile.TileContext(nc) as tc, ExitStack() as ctx:
        # ---- persistent SBUF pools ----
        single = ctx.enter_context(tc.tile_pool(name="single", bufs=1))
        xpool = ctx.enter_context(tc.tile_pool(name="xres", bufs=2))
        hpool = ctx.enter_context(tc.tile_pool(name="hln", bufs=2))
        qkpool = ctx.enter_context(tc.tile_pool(name="qk", bufs=1))
        vpool = ctx.enter_context(tc.tile_pool(name="v", bufs=1))
        opool = ctx.enter_context(tc.tile_pool(name="oatt", bufs=1))
        ppool = ctx.enter_context(tc.tile_pool(name="pprob", bufs=6))
        statp = ctx.enter_context(tc.tile_pool(name="stat", bufs=1))
        sqpool = ctx.enter_context(tc.tile_pool(name="sq", bufs=3))
        srpool = ctx.enter_context(tc.tile_pool(name="sr", bufs=2))
        apool = ctx.enter_context(tc.tile_pool(name="agelu", bufs=2))
        w3pool = ctx.enter_context(tc.tile_pool(name="w3", bufs=2))
        bigw = ctx.enter_context(tc.tile_pool(name="bigw", bufs=1))
        bigw2 = ctx.enter_context(tc.tile_pool(name="bigw2", bufs=1))

        ones_sb = single.tile([128, 128], F32R, tag="ones")
        nc.sync.dma_start(out=ones_sb[:], in_=dr["ones"].ap())
        eps_sb = single.tile([128, 1], F32, tag="eps")
        nc.vector.memset(eps_sb[:], EPS)

        pos_sb = bigw.tile([128, FT, NTOK], F32R, tag="fc1w")
        nc.sync.dma_start(out=pos_sb[:], in_=dr["pos"].ap())
        cls_sb = single.tile([128, FT], F32R, tag="cls")
        nc.sync.dma_start(out=cls_sb[:], in_=dr["cls"].ap())

        lnS = {}
        if not triv["ln1"]:
            lnS["l1s"] = single.tile([128, L, FT], F32R, tag="l1s")
            lnS["l1b"] = single.tile([128, L, FT], F32R, tag="l1b")
            nc.sync.dma_start(out=lnS["l1s"][:], in_=dr["ln1s"].ap())
            nc.sync.dma_start(out=lnS["l1b"][:], in_=dr["ln1b"].ap())
        if not triv["ln2"]:
            lnS["l2s"] = single.tile([128, L, FT], F32R, tag="l2s")
            lnS["l2b"] = single.tile([128, L, FT], F32R, tag="l2b")
            nc.sync.dma_start(out=lnS["l2s"][:], in_=dr["ln2s"].ap())
            nc.sync.dma_start(out=lnS["l2b"][:], in_=dr["ln2b"].ap())
        biases = {}
        for bn, n1 in [("qkv_b", 9), ("proj_b", FT), ("fc1_b", FKT), ("fc2_b", FT)]:
            if not triv[bn]:
                biases[bn] = single.tile([128, L, n1], F32, tag=bn)
                nc.sync.dma_start(out=biases[bn][:], in_=dr[bn].ap())
        for bn, n1 in [("patch_b", FT), ("head_b1", 16), ("head_b2", 2)]:
            if not triv[bn]:
                biases[bn] = single.tile([128, n1], F32, tag=bn)
                nc.sync.dma_start(out=biases[bn][:], in_=dr[bn].ap())
        vb_sb = None
        if not triv["qkv_b"]:
            vb_sb = single.tile([1, L, D], F32R, tag="vb")
            nc.sync.dma_start(out=vb_sb[:], in_=dr["qkv_bv"].ap())

        def psum_copy(dst_ap, src_ap, bias_ap=None, eng=None):
            """PSUM -> SBUF move, optionally adding a per-partition bias."""
            if bias_ap is None:
                if eng == "act":
                    nc.scalar.copy(dst_ap, src_ap)
                else:
                    nc.vector.tensor_copy(dst_ap, src_ap)
            else:
                nc.vector.tensor_scalar_add(dst_ap, src_ap, bias_ap)

        # ---------------- patch embed + cls + pos ----------------
        x_t = xpool.tile([128, FT, T], F32R, tag="x")
        with tc.tile_pool(name="ps_patch", bufs=3, space="PSUM") as psp:
            pb = biases.get("patch_b")
            for i in range(NI):
                # cls token column
                nc.vector.tensor_tensor(
                    out=x_t[:, :, i * NTOK:i * NTOK + 1],
                    in0=cls_sb[:].unsqueeze(2),
                    in1=pos_sb[:, :, 0:1],
                    op=OP.add)
            wpat = []
            for mt in range(FT):
                w = w3pool.tile([128, 6, 128], F32R, tag="wpatch", bufs=3,
                                name=f"patchw_{mt}")
                nc.sync.dma_start(out=w[:], in_=dr["patchw"].ap()[mt])
                wpat.append(w)
            for i in range(NI):
                for (c0, csz) in PCH:
                    rhs = bigw2.tile([128, 6, csz], F32R, tag="fc2w", bufs=1,
                                     name=f"prhs_{i}_{c0}")
                    nc.sync.dma_start(
                        out=rhs[:],
                        in_=dr["patches"].ap()[:, :, i * NPAT + c0:i * NPAT + c0 + csz])
                    for mt in range(FT):
                        ps = psp.tile([128, csz], F32, tag="mm")
                        for kt in range(6):
                            nc.tensor.matmul(ps[:], wpat[mt][:, kt, :], rhs[:, kt, :],
                                             start=(kt == 0), stop=(kt == 5))
                        dst = x_t[:, mt, i * NTOK + 1 + c0:i * NTOK + 1 + c0 + csz]
                        pos_sl = pos_sb[:, mt, 1 + c0:1 + c0 + csz]
                        if pb is None:
                            nc.vector.tensor_tensor(out=dst, in0=ps[:], in1=pos_sl, op=OP.add)
                        else:
                            nc.vector.scalar_tensor_tensor(
                                out=dst, in0=ps[:], scalar=pb[:, mt], in1=pos_sl,
                                op0=OP.add, op1=OP.add)

        # ---------------- transformer layers ----------------
        _ln_uid = [0]

        def emit_ln(src, dst, s_ap, b_ap):
            """dst = LN(src) over the feature (partition-tiled) axis.
            src/dst: [128, FT, T] feature-major tiles. Fully chunk-granular so
            each chunk of dst unblocks downstream consumers early (cross-phase
            pipelining); stats via ones-matmuls (free 128-partition broadcast),
            squares on the otherwise-idle GPSIMD, rstd = exp(-0.5*ln(var))."""
            _ln_uid[0] += 1
            uid = _ln_uid[0]
            with tc.tile_pool(name="ps_ln", bufs=6, space="PSUM") as pln:
                m_b = statp.tile([128, T], F32, tag="m", name=f"lnm_{uid}")
                r_b = statp.tile([128, T], F32, tag="r", name=f"lnr_{uid}")
                for ci, (c0, csz) in enumerate(CH):
                    s1 = pln.tile([128, csz], F32, tag="ln", name=f"s1_{uid}_{ci}")
                    s2 = pln.tile([128, csz], F32, tag="ln", name=f"s2_{uid}_{ci}")
                    for ft in range(FT):
                        sl = src[:, ft, c0:c0 + csz]
                        sq = sqpool.tile([128, csz], F32R, tag="sq",
                                         name=f"sq_{uid}_{ci}_{ft}")
                        nc.gpsimd.tensor_mul(sq[:], sl, sl)
                        nc.tensor.matmul(s1[:], ones_sb[:], sl,
                                         start=(ft == 0), stop=(ft == FT - 1))
                        nc.tensor.matmul(s2[:], ones_sb[:], sq[:],
                                         start=(ft == 0), stop=(ft == FT - 1))
                    mc = m_b[:, c0:c0 + csz]
                    rc = r_b[:, c0:c0 + csz]
                    nc.vector.tensor_scalar_mul(mc, s1[:], 1.0 / D)
                    t2 = sqpool.tile([128, csz], F32, tag="sq", name=f"t2_{uid}_{ci}")
                    nc.gpsimd.tensor_mul(t2[:], mc, mc)
                    t1 = sqpool.tile([128, csz], F32, tag="sq", name=f"t1_{uid}_{ci}")
                    # var = s2/D - m^2; +eps folded into Ln's free bias
                    nc.vector.scalar_tensor_tensor(
                        out=t1[:], in0=s2[:], scalar=1.0 / D, in1=t2[:],
                        op0=OP.mult, op1=OP.subtract)
                    nc.scalar.activation(t1[:], t1[:], AF.Ln, bias=eps_sb[:])
                    nc.scalar.activation(rc, t1[:], AF.Exp, scale=-0.5)
                    for ft in range(FT):
                        dsl = dst[:, ft, c0:c0 + csz]
                        eng = nc.gpsimd if ft == 2 else nc.vector
                        eng.tensor_sub(dsl, src[:, ft, c0:c0 + csz], mc)
                        eng.tensor_mul(dsl, dsl, rc)
                        if s_ap is not None:
                            nc.vector.tensor_scalar(dsl, dsl, s_ap[:, ft], b_ap[:, ft],
                                                    op0=OP.mult, op1=OP.add)

        x_cur = x_t
        for l in range(L):
            # ---- LN1 ----
            h_t = hpool.tile([128, FT, T], F32R, tag="h")
            sA = lnS["l1s"][:, l, :] if not triv["ln1"] else None
            bA = lnS["l1b"][:, l, :] if not triv["ln1"] else None
            emit_ln(x_cur, h_t, sA, bA)

            # ---- qkv ----
            qk_t = qkpool.tile([128, QKT, T], F32R, tag="qk")
            v_t = vpool.tile([128, 2 * 5, NH, HD + 1], F32R, tag="v")
            with tc.tile_pool(name="ps_qkv", bufs=4, space="PSUM") as pq:
                qbias = biases.get("qkv_b")
                for g in range(2):
                    w = w3pool.tile([128, FT, 3 * 128], F32R, tag="w3b",
                                    bufs=2, name=f"qkvw_{l}_{g}")
                    nc.sync.dma_start(out=w[:], in_=dr["qkvw"].ap()[l, g])
                    for ms in range(3):
                        mt = g * 3 + ms
                        for (c0, csz) in CH:
                            ps = pq.tile([128, csz], F32, tag="mm")
                            for ft in range(FT):
                                nc.tensor.matmul(
                                    ps[:], w[:, ft, ms * 128:(ms + 1) * 128],
                                    h_t[:, ft, c0:c0 + csz],
                                    start=(ft == 0), stop=(ft == FT - 1))
                            psum_copy(qk_t[:, mt, c0:c0 + csz], ps[:],
                                      qbias[:, l, mt] if qbias is not None else None,
                                      eng=None)
                # v in token-major [tok, head, hd] with a fused ones column
                vr = w3pool.tile([128, FT, D], F32R, tag="vr", name=f"vr_{l}")
                nc.sync.dma_start(out=vr[:], in_=dr["qkvwv"].ap()[l])
                for i in range(NI):
                    for mi, (m0, msz) in enumerate(AMT):
                        g0 = i * NTOK + m0
                        ps = pq.tile([128, D], F32, tag="mm")
                        for ft in range(FT):
                            nc.tensor.matmul(ps[0:msz, :], h_t[:, ft, g0:g0 + msz],
                                             vr[:, ft, :], start=(ft == 0),
                                             stop=(ft == FT - 1 and vb_sb is None))
                        if vb_sb is not None:
                            nc.tensor.matmul(ps[0:msz, :], ones_sb[0:1, 0:msz],
                                             vb_sb[0:1, l, :], start=False, stop=True)
                        vdst = v_t[0:msz, i * 5 + mi, :, 0:HD]
                        vsrc = ps[0:msz, :].rearrange("p (h d) -> p h d", h=NH)
                        nc.vector.tensor_copy(vdst, vsrc)
                        nc.vector.tensor_copy(
                            v_t[0:msz, i * 5 + mi, :, HD:HD + 1],
                            ones_sb[0:msz, 0:NH].unsqueeze(2))

            # ---- attention ----
            # S^T computed per (img, head) into bank-aligned mega-PSUM tiles
            # ([128, 5 m-tiles, 512] + [128, 5, 65]) so exp is 4 ACT ops per
            # pair instead of 10, amortizing ACT's fixed per-op cost.
            o_t = opool.tile([128, FT, T], F32R, tag="o")
            with tc.tile_pool(name="ps_attn", bufs=1, space="PSUM") as pa, \
                 tc.tile_pool(name="ps_o", bufs=4, space="PSUM") as po:
                for i in range(NI):
                    for hh in range(NH):
                        qoff = 64 * (hh % 2)
                        qt = hh // 2
                        ktile = 3 + hh // 2
                        base = i * NTOK
                        # One [128, 2, 512] psum tile per m-tile holds both
                        # n-chunks bank-aligned: every matmul keeps a >=256
                        # moving dim (full fp32r rate) and S tiles pipeline
                        # at m-tile granularity (bufs=3).
                        pts = []
                        for mi, (m0, msz) in enumerate(AMT):
                            gm = base + m0
                            lhs = qk_t[qoff:qoff + HD, ktile, gm:gm + msz]
                            sps = pa.tile([128, 2, 512], F32, tag="s2", bufs=2,
                                          name=f"s_{l}_{i}_{hh}_{mi}")
                            for ci, (n0, nsz) in enumerate(ACH):
                                nc.tensor.matmul(
                                    sps[0:msz, ci, 0:nsz], lhs,
                                    qk_t[qoff:qoff + HD, qt,
                                         base + n0:base + n0 + nsz],
                                    start=True, stop=True)
                            pt = ppool.tile([128, 2, 290], F32R, tag="p", bufs=8,
                                            name=f"p_{l}_{i}_{hh}_{mi}")
                            # single exp over both chunks; the strided view's
                            # dead columns (slot 1, cols 288-289) are unread
                            nc.scalar.activation(pt[0:msz, :, :],
                                                 sps[0:msz, :, 0:290],
                                                 AF.Exp, scale=ATTN_SCALE)
                            pts.append(pt)
                        for ci, (n0, nsz) in enumerate(ACH):
                            gn = base + n0
                            ops = po.tile([128, nsz], F32, tag="o",
                                          name=f"ops_{l}_{i}_{hh}_{ci}")
                            for mi, (m0, msz) in enumerate(AMT):
                                nc.tensor.matmul(
                                    ops[0:HD + 1, :],
                                    v_t[0:msz, i * 5 + mi, hh, :],
                                    pts[mi][0:msz, ci, 0:nsz],
                                    start=(mi == 0), stop=(mi == len(AMT) - 1))
                            sr = srpool.tile([128, nsz], F32R, tag="sr",
                                             name=f"sr_{l}_{i}_{hh}_{ci}")
                            nc.vector.tensor_copy(sr[64:65, :], ops[64:65, :])
                            bc = po.tile([128, nsz], F32, tag="o",
                                         name=f"bc_{l}_{i}_{hh}_{ci}")
                            nc.tensor.matmul(bc[0:64, :], ones_sb[64:65, 0:64],
                                             sr[64:65, :], start=True, stop=True)
                            rec = srpool.tile([128, nsz], F32, tag="rec",
                                              name=f"rec_{l}_{i}_{hh}_{ci}")
                            nc.vector.reciprocal_approx_fast(
                                out=rec[0:64, :], in_=bc[0:64, :])
                            nc.vector.tensor_tensor(
                                out=o_t[qoff:qoff + HD, hh // 2, gn:gn + nsz],
                                in0=ops[0:HD, :], in1=rec[0:64, :], op=OP.mult)

            # ---- proj + residual ----
            x_new = xpool.tile([128, FT, T], F32R, tag="x")
            with tc.tile_pool(name="ps_proj", bufs=3, space="PSUM") as pp:
                pbias = biases.get("proj_b")
                wpj = w3pool.tile([128, FT, 3 * 128], F32R, tag="w3b",
                                  bufs=2, name=f"projw_{l}")
                nc.sync.dma_start(out=wpj[:], in_=dr["projw"].ap()[l, 0])
                for mt in range(FT):
                    for (c0, csz) in CHI:
                        ps = pp.tile([128, csz], F32, tag="mm")
                        for ft in range(FT):
                            nc.tensor.matmul(ps[:],
                                             wpj[:, ft, mt * 128:(mt + 1) * 128],
                                             o_t[:, ft, c0:c0 + csz],
                                             start=(ft == 0), stop=(ft == FT - 1))
                        dst = x_new[:, mt, c0:c0 + csz]
                        if pbias is None:
                            nc.vector.tensor_tensor(dst, ps[:],
                                                    x_cur[:, mt, c0:c0 + csz], op=OP.add)
                        else:
                            nc.vector.scalar_tensor_tensor(
                                out=dst, in0=ps[:], scalar=pbias[:, l, mt],
                                in1=x_cur[:, mt, c0:c0 + csz], op0=OP.add, op1=OP.add)
            x_cur = x_new

            # ---- LN2 ----
            h2 = hpool.tile([128, FT, T], F32R, tag="h")
            sA = lnS["l2s"][:, l, :] if not triv["ln2"] else None
            bA = lnS["l2b"][:, l, :] if not triv["ln2"] else None
            emit_ln(x_cur, h2, sA, bA)

            # ---- MLP: fc1 -> gelu -> fc2 + residual ----
            f1bias = biases.get("fc1_b")
            f2bias = biases.get("fc2_b")
            x_out = xpool.tile([128, FT, T], F32R, tag="x")
            with tc.tile_pool(name="ps_mlp", bufs=3, space="PSUM") as pm, \
                 tc.tile_pool(name="ps_f2", bufs=3, space="PSUM") as pf2:
                for (c0, csz) in CH:
                    accs = [pf2.tile([128, csz], F32, tag="f2acc",
                                     name=f"f2acc_{l}_{c0}_{mt}")
                            for mt in range(FT)]
                    for fg in range(FKT // 3):
                        f1wt = w3pool.tile([128, FT, 3 * 128], F32R, tag="w3b",
                                           bufs=2, name=f"f1w_{l}_{c0}_{fg}")
                        nc.sync.dma_start(
                            out=f1wt[:],
                            in_=dr["fc1w"].ap()[l][:, :, fg * 384:(fg + 1) * 384])
                        f2wt = w3pool.tile([128, 3, D], F32R, tag="fc2r",
                                           bufs=2, name=f"f2w_{l}_{c0}_{fg}")
                        nc.sync.dma_start(
                            out=f2wt[:],
                            in_=dr["fc2w"].ap()[l][:, fg * 3:(fg + 1) * 3, :])
                        for fs in range(3):
                            fk = fg * 3 + fs
                            f1ps = pm.tile([128, csz], F32, tag="fc1")
                            for ft in range(FT):
                                nc.tensor.matmul(f1ps[:],
                                                 f1wt[:, ft, fs * 128:(fs + 1) * 128],
                                                 h2[:, ft, c0:c0 + csz],
                                                 start=(ft == 0), stop=(ft == FT - 1))
                            a_t = apool.tile([128, csz], F32R, tag="a")
                            if f1bias is None:
                                nc.scalar.activation(a_t[:], f1ps[:], AF.Gelu)
                            else:
                                nc.scalar.activation(a_t[:], f1ps[:], AF.Gelu,
                                                     bias=f1bias[:, l, fk])
                            for mt in range(FT):
                                nc.tensor.matmul(accs[mt][:],
                                                 f2wt[:, fs, mt * 128:(mt + 1) * 128],
                                                 a_t[:], start=(fk == 0),
                                                 stop=(fk == FKT - 1))
                    for mt in range(FT):
                        dst = x_out[:, mt, c0:c0 + csz]
                        if f2bias is None:
                            nc.vector.tensor_tensor(dst, accs[mt][:],
                                                    x_cur[:, mt, c0:c0 + csz], op=OP.add)
                        else:
                            nc.vector.scalar_tensor_tensor(
                                out=dst, in0=accs[mt][:], scalar=f2bias[:, l, mt],
                                in1=x_cur[:, mt, c0:c0 + csz], op0=OP.add, op1=OP.add)
            x_cur = x_out

        # ---------------- final LN on cls columns + head ----------------
        # cls tokens are columns 0 and 577 of x
        cview = x_cur[:, :, :].rearrange("p f (i n) -> p f i n", n=NTOK)[:, :, :, 0]
        c_ln = single.tile([128, FT, NI], F32R, tag="cln")
        with tc.tile_pool(name="ps_fin", bufs=4, space="PSUM") as pf:
            s1 = pf.tile([128, NI], F32, tag="ln")
            s2 = pf.tile([128, NI], F32, tag="ln")
            sqc = single.tile([128, FT, NI], F32R, tag="sqc")
            for ft in range(FT):
                nc.scalar.activation(sqc[:, ft, :], cview[:, ft, :], AF.Square)
                nc.tensor.matmul(s1[:], ones_sb[:], cview[:, ft, :],
                                 start=(ft == 0), stop=(ft == FT - 1))
                nc.tensor.matmul(s2[:], ones_sb[:], sqc[:, ft, :],
                                 start=(ft == 0), stop=(ft == FT - 1))
            m_b = statp.tile([128, NI], F32, tag="m")
            nc.vector.tensor_scalar_mul(m_b[:], s1[:], 1.0 / D)
            t1 = statp.tile([128, NI], F32, tag="t1")
            nc.vector.tensor_scalar(t1[:], s2[:], 1.0 / D, EPS, op0=OP.mult, op1=OP.add)
            t2 = statp.tile([128, NI], F32, tag="r")
            nc.vector.tensor_mul(t2[:], m_b[:], m_b[:])
            nc.vector.tensor_sub(t1[:], t1[:], t2[:])
            nc.scalar.activation(t1[:], t1[:], AF.Ln)
            nc.scalar.activation(t1[:], t1[:], AF.Exp, scale=-0.5)
            for ft in range(FT):
                nc.vector.tensor_sub(c_ln[:, ft, :], cview[:, ft, :], m_b[:])
                nc.vector.tensor_mul(c_ln[:, ft, :], c_ln[:, ft, :], t1[:])
                if not triv["norm"]:
                    ns = single.tile([128, FT], F32R, tag="ns")
                    nb = single.tile([128, FT], F32R, tag="nb")
                    if ft == 0:
                        nc.sync.dma_start(out=ns[:], in_=dr["norms"].ap())
                        nc.sync.dma_start(out=nb[:], in_=dr["normb"].ap())
                    nc.vector.tensor_scalar(c_ln[:, ft, :], c_ln[:, ft, :],
                                            ns[:, ft], nb[:, ft],
                                            op0=OP.mult, op1=OP.add)

            # head: relu(w1 @ cls) -> w2 @ .
            h1_t = single.tile([128, 16, NI], F32R, tag="h1")
            hb1 = biases.get("head_b1")
            for mt in range(16):
                w = w3pool.tile([128, FT, 128], F32R, tag="w3")
                nc.sync.dma_start(out=w[:], in_=dr["headw1"].ap()[mt])
                ps = pf.tile([128, NI], F32, tag="hmm")
                for ft in range(FT):
                    nc.tensor.matmul(ps[:], w[:, ft, :], c_ln[:, ft, :],
                                     start=(ft == 0), stop=(ft == FT - 1))
                if hb1 is None:
                    nc.scalar.activation(h1_t[:, mt, :], ps[:], AF.Relu)
                else:
                    nc.scalar.activation(h1_t[:, mt, :], ps[:], AF.Relu,
                                         bias=hb1[:, mt])
            out_sb = single.tile([128, 2, NI], F32, tag="osb")
            hb2 = biases.get("head_b2")
            for mt in range(2):
                w2 = bigw.tile([128, 16, 128], F32R, tag="fc1w")
                nc.sync.dma_start(out=w2[:], in_=dr["headw2"].ap()[mt])
                ps = pf.tile([128, NI], F32, tag="hmm")
                for kt in range(16):
                    nc.tensor.matmul(ps[:], w2[:, kt, :], h1_t[:, kt, :],
                                     start=(kt == 0), stop=(kt == 15))
                psum_copy(out_sb[:, mt, :], ps[:],
                          hb2[:, mt] if hb2 is not None else None)
            nc.sync.dma_start(
                out=out_d.ap().rearrange("(mt p) c -> p mt c", p=128),
                in_=out_sb[:])

    if compile:
        nc.compile()
    return nc


_CACHE = {}


def _get_program(triv):
    key = tuple(sorted(triv.items()))
    if key not in _CACHE:
        _CACHE[key] = _build(triv)
    return _CACHE[key]


def kernel(**inputs) -> np.ndarray:
    d, triv = _host_prep(inputs)
    nc = _get_program(triv)

    common = {}
    for k in ("patchw", "pos", "cls", "qkvw", "qkvwv",
              "projw", "fc1w", "fc2w", "headw1", "headw2", "ones"):
        common[k] = d[k]
    if not triv["ln1"]:
        common["ln1s"], common["ln1b"] = d["ln1s"], d["ln1b"]
    if not triv["ln2"]:
        common["ln2s"], common["ln2b"] = d["ln2s"], d["ln2b"]
    if not triv["norm"]:
        common["norms"], common["normb"] = d["norms"], d["normb"]
    for bn in ("qkv_b", "proj_b", "fc1_b", "fc2_b", "patch_b", "head_b1", "head_b2"):
        if not triv[bn]:
            common[bn] = d[bn]
    if not triv["qkv_b"]:
        common["qkv_bv"] = d["qkv_bv"]

    in_maps = [dict(common, patches=d["patches"][c]) for c in range(NCORES)]
    res = bass_utils.run_bass_kernel_spmd(nc, in_maps, core_ids=list(range(NCORES)))

    out = np.zeros((B, 256), np.float32)
    for c in range(NCORES):
        oc = np.asarray(res.results[c]["out"], np.float32)   # [256, NI]
        out[c * NI:(c + 1) * NI, :] = oc.T
    return out


if __name__ == "__main__":
    import os, time
    triv = dict(ln1=True, ln2=True, norm=True, qkv_b=True, proj_b=True,
                fc1_b=True, fc2_b=True, patch_b=True, head_b1=True, head_b2=True)
    do_compile = os.environ.get("KERNEL_COMPILE", "0") == "1"
    t0 = time.time()
    nc = _build(triv, compile=do_compile)
    print("build s:", time.time() - t0, "compile:", do_compile)
    print("instructions:", sum(len(b.instructions) for b in nc.m.functions[0].blocks))
    from concourse.timeline_sim import TimelineSim
    ts = TimelineSim(nc, trace=False)
    dur = ts.simulate()
    print("TimelineSim duration:", dur, "ns")
